# revision 1
# baseline (speedup 1.0000x reference)
"""Bass/Trainium2 kernel for nn_AdaptiveSparseReservoir (self-contained).

out[b, c] = relu(sum_k x[b, rows[k]] * values[k] for cols[k]==c  + bias[c])
  x [1024, 4096] f32; values [262144] f32; rows/cols [262144] i32;
  bias [4096] f32  ->  out [1024, 4096] f32

Strategy
--------
Densify the sparse COO kernel on the host into W [4096, 4096] (1.6%
density with unstructured support is far too dense for gather/scatter on
TRN2 — a dense bf16 TensorEngine matmul moves ~16x fewer bytes), then run
the dense matmul column-sharded across the 8 NeuronCores with NO
collectives: core i computes outT_i = relu(W[:, 512i:512(i+1)].T @ x.T + b_i).

Measured-on-silicon design points:
- PSUM-accumulating bf16 matmuls (K=128, N=512) retire at ~213-226 ns —
  one rhs column per 2.4 GHz cycle is the architectural floor (~55 us for
  the 256 matmuls/core; fp8 DoubleRow would halve it but its ~5% quant
  error fails the 2e-2 gate; no int8 matmul path exists on TRN2 bass).
  DMA (12 MB/core) streams at ~310 GB/s split across BOTH HWDGE rings
  (sync + scalar) and hides under the matmul stream: "ridge" regime.
- The output is computed TRANSPOSED so the per-column bias lands on the
  PSUM partition axis: bias+relu is then a single fused op per PSUM bank,
  alternating ScalarE `activation` / VectorE `tensor_scalar`.
- STAGGERED DRAIN (-3 us/body vs the clustered-epilogue baseline, median
  of eight interleaved A/B sessions): the last 4 k-tiles run
  bank-pair-major, so bank completions spread over the final ~6.8 us of
  the matmul stream and every bias+relu epilogue + out-DMA pipelines
  UNDER the stream instead of serializing after it. The second-to-last
  bank's halved epilogue hides under the last bank's matmuls; the last
  bank's epilogue is QUARTERED across both engines so its first 32 KB
  out-DMA launches ~200 ns after the final matmul, two quarters per
  ring in parallel. PSUM collisions are per-BANK (an epilogue read
  blocks further matmuls to that bank), so this is the floor at bank
  granularity.
- EARLY START: the DMA head is split so the first matmul — an N=256 half
  of bank (0,0) — waits only on a 64 KB x chunk + 32 KB w chunk on
  parallel rings (~0.5 us). start=True clears has_written for the WHOLE
  bank, so only the first half sets it; the second half runs start=False
  and overwrites its cleared region (verified on silicon). The chunk head
  is k-granular (no PE stall in an arrival-vs-consumption model even at
  120 GB/s/ring); the bias load rides behind the stream. Short N=128
  warm-up matmuls cover the HAM cold-clock window until data lands.
- TileContext's exit barrier is replaced by a drain-only tail: the Bass
  preamble sem_clears at the start of every execution, so the butterfly
  barrier + semaphore clears (~4 us) are dead weight.
- Ruled out on silicon: coarse DMA chunking with greedy ring balancing
  (+712 ns, descriptor count immaterial) and 4-k-contiguous bank-major
  matmul order (+1020 ns, psum bank round-robin costs nothing here).
"""

import types

import numpy as np
import ml_dtypes

D_IN = 4096
UNITS = 4096
NNZ = 262144
BATCH = 1024
N_CORES = 8
N_SHARD = UNITS // N_CORES  # 512 output columns per core
K_TILES = D_IN // 128  # 32
N_TILES = N_SHARD // 128  # 4
M_HALVES = BATCH // 512  # 2

_CACHE = {}


def _drain_only(self, tick_clock, wait_clock):
    """Tail = DMA/compute drain only; skip the butterfly barrier + sem
    clears (the Bass preamble sem_clears at the start of each execution,
    and NEFF completion already requires every engine queue to finish)."""
    from concourse.tile import ScopedClock

    drain_inst = self.nc.sync.drain()
    wait_clock.add_sem_waits(
        drain_inst.ins, ScopedClock({None: tick_clock.global_clock})
    )
    popped = self.nc._tile_sem_poison_stack.pop()
    assert popped is self._sem_poison


def _build(reps=1):
    import concourse.mybir as mybir
    import concourse.tile as tile
    from concourse import bacc

    nc = bacc.Bacc("TRN2", target_bir_lowering=False, debug=False, num_devices=N_CORES)
    bf16 = mybir.dt.bfloat16
    f32 = mybir.dt.float32

    xT_ext = nc.declare_dram_parameter("xT", [128, K_TILES * 1024], bf16, isOutput=False)
    w_ext = nc.declare_dram_parameter("w", [128, K_TILES * 512], bf16, isOutput=False)
    b_ext = nc.declare_dram_parameter("bias", [128, N_TILES], f32, isOutput=False)
    out_ext = nc.declare_dram_parameter("out", [N_SHARD, BATCH], bf16, isOutput=True)

    tc_outer = tile.TileContext(nc)
    try:
        # verify the internals _drain_only touches exist in this concourse
        from concourse.tile import ScopedClock  # noqa: F401

        assert hasattr(tc_outer, "_drain_and_barrier")
        assert hasattr(nc, "_tile_sem_poison_stack")
        tc_outer._drain_and_barrier = types.MethodType(_drain_only, tc_outer)
    except Exception:
        pass  # stock barrier exit: ~4us slower, still correct
    with tc_outer as tc:
        with (
            tc.tile_pool(name="consts", bufs=1) as cpool,
            tc.tile_pool(name="xk", bufs=1) as xpool,
            tc.tile_pool(name="wk", bufs=1) as wpool,
            tc.tile_pool(name="osb", bufs=9) as opool,
            tc.tile_pool(name="psum", bufs=1, space="PSUM") as ppool,
        ):
            psum = [
                ppool.tile([128, 512], f32, tag=f"ps{i}", name=f"ps{i}")
                for i in range(N_TILES * M_HALVES)
            ]

            # PE warm-up against the HAM cold clock: short N=128 matmuls keep
            # the PE-busy window covered until the first data chunk lands,
            # with at most ~107 ns of overshoot past data arrival (an N=512
            # warm-up would block the queue 427 ns at the cold rate). k=0's
            # start=True clear discards the garbage.
            # NO memset — the warm-ups read a RAW (untracked, uninitialized)
            # SBUF tensor and start with zero dependencies (~0.2 us earlier
            # PE-busy, pulling the HAM un-throttle point earlier). Any NaN
            # garbage lands in psum bank 0 whose has_written bits the first
            # real start=True matmul clears; cleared bits force overwrite,
            # so garbage is never read (the same data-independent semantics
            # the split k=0 bank relies on, verified on silicon).
            warm = nc.alloc_sbuf_tensor("warm_raw", [128, 128], bf16)
            for _ in range(5):
                # lhsT and rhs may share the region: both are SBUF reads
                nc.tensor.matmul(
                    psum[0][:, :128], warm[:, 0:128], warm[:, 0:128],
                    start=True, stop=True,
                )

            tbl_warm = cpool.tile([128, 1], f32)
            bias_sb = cpool.tile([128, N_TILES], f32)

            xts = xpool.tile([128, K_TILES * 1024], bf16, name="xts")
            wts = wpool.tile([128, K_TILES * 512], bf16, name="wts")

            # interleave x/w chunks in k order, alternating HWDGE rings;
            # fine-grained first chunks (early PE start). The head is split
            # so the FIRST matmul — an N=256 half of bank (0,0) — waits only
            # on a 64 KB x transfer and a 32 KB w transfer on parallel
            # rings (~0.5 us), instead of two 128 KB ones.
            chunks = [
                ("x", 0, 256), ("w", 0, 128), ("x", 256, 512),
                ("w", 128, 512), ("x", 512, 1024), ("w", 512, 1024),
            ]
            xbounds = [1, 2, 3, 4, 5, 6] + list(range(8, K_TILES + 1, 2))
            wbounds = [2, 3, 4, 6, 8] + list(range(12, K_TILES + 1, 4))
            xi = wi = 0
            while xi < len(xbounds) - 1 or wi < len(wbounds) - 1:
                kx = xbounds[xi] if xi < len(xbounds) - 1 else K_TILES
                kw = wbounds[wi] if wi < len(wbounds) - 1 else K_TILES
                if kw < kx and wi < len(wbounds) - 1:
                    chunks.append(("w", wbounds[wi] * 512, wbounds[wi + 1] * 512))
                    wi += 1
                else:
                    chunks.append(("x", xbounds[xi] * 1024, xbounds[xi + 1] * 1024))
                    xi += 1

            def mm(k, nt, mh, stop=False):
                nc.tensor.matmul(
                    psum[nt * M_HALVES + mh][:, :],
                    wts[:, k * 512 + nt * 128 : k * 512 + (nt + 1) * 128],
                    xts[:, k * 1024 + mh * 512 : k * 1024 + (mh + 1) * 512],
                    start=(k == 0),
                    stop=stop,
                )

            def epilogue(r, i, nt, mh, lo, hi, on_scalar):
                ot = opool.tile([128, hi - lo], bf16, name=f"ot{r}_{i}_{lo}", tag="ot")
                if on_scalar:
                    nc.scalar.activation(
                        ot[:, :],
                        psum[nt * M_HALVES + mh][:, lo:hi],
                        mybir.ActivationFunctionType.Relu,
                        bias=bias_sb[:, nt : nt + 1],
                    )
                else:
                    nc.vector.tensor_scalar(
                        ot[:, :],
                        psum[nt * M_HALVES + mh][:, lo:hi],
                        bias_sb[:, nt : nt + 1],
                        0.0,
                        mybir.AluOpType.add,
                        mybir.AluOpType.max,
                    )
                return ot

            K_STAG = K_TILES - 4
            for r in range(reps):
                for i, (kind, clo, chi) in enumerate(chunks):
                    eng = nc.sync if i % 2 == 0 else nc.scalar
                    if kind == "x":
                        eng.dma_start(xts[:, clo:chi], xT_ext[:, clo:chi])
                    else:
                        eng.dma_start(wts[:, clo:chi], w_ext[:, clo:chi])

                if r == 0:
                    # bias (2 KB) rides behind the input stream — needed
                    # only by the first epilogue ~6 us before stream end
                    nc.sync.dma_start(bias_sb[:, :], b_ext[:, :])
                    # trigger the Relu act-table load now (ACT is idle during
                    # the stream); bacc hoists LoadActFuncSet before this
                    # instruction, keeping the ~1.3us load off the epilogue
                    # critical path
                    nc.scalar.activation(
                        tbl_warm[:, :], warm[:, 0:1],
                        mybir.ActivationFunctionType.Relu,
                    )

                # k=0, mh=0 first across all nt (only x cols 0:512 needed);
                # bank (0,0) starts as two N=256 halves gated on just the
                # first 64 KB x chunk. start=True clears has_written for the
                # WHOLE bank, so only the first half sets it; the second
                # half runs start=False and overwrites its cleared region.
                nc.tensor.matmul(
                    psum[0][:, 0:256], wts[:, 0:128], xts[:, 0:256],
                    start=True, stop=False,
                )
                nc.tensor.matmul(
                    psum[0][:, 256:512], wts[:, 0:128], xts[:, 256:512],
                    start=False, stop=False,
                )
                for nt in range(1, N_TILES):
                    mm(0, nt, 0)
                for nt in range(N_TILES):
                    mm(0, nt, 1)

                # phase 1: k-major over all banks — keeps the PE stream dense
                # while DMA feeds k-tiles. mh-inner so each LDWEIGHTS serves
                # two matmuls.
                for k in range(1, K_STAG):
                    for nt in range(N_TILES):
                        for mh in range(M_HALVES):
                            mm(k, nt, mh)

                # phase 2: bank-pair-major — each nt runs its last 4 k-tiles
                # (mh pairs share LDWEIGHTS) then both banks drain (fused
                # bias+relu, alternating ScalarE/VectorE; out-DMA alternating
                # rings). Pair completions are staggered 8 MMs (~1.7 us)
                # apart, so all 8 epilogues + out-DMAs pipeline UNDER the
                # remaining matmul stream instead of serializing after it.
                # The final pair's epilogues are split into halves across
                # both engines and both rings to shorten the exposed tail.
                for nt in range(N_TILES - 1):
                    for k in range(K_STAG, K_TILES):
                        mm(k, nt, 0, stop=(k == K_TILES - 1))
                        mm(k, nt, 1, stop=(k == K_TILES - 1))
                    for mh in range(M_HALVES):
                        i = nt * M_HALVES + mh
                        orow = out_ext[
                            nt * 128 : (nt + 1) * 128, mh * 512 : (mh + 1) * 512
                        ]
                        ot = epilogue(r, i, nt, mh, 0, 512, on_scalar=(i % 2 == 0))
                        eng = nc.sync if i % 2 == 0 else nc.scalar
                        eng.dma_start(orow, ot[:, :])

                # final pair (nt=3): each bank's epilogue is halved across
                # ACT (lo) and DVE (hi), and the four 64 KB out-DMAs are
                # balanced lo->sync / hi->scalar so both rings drain two each
                # in parallel. PSUM collisions are per-bank, so nothing can
                # start a bank's epilogue before its last matmul — this is
                # the minimal exposed tail at bank granularity.
                nt = N_TILES - 1
                # bank (nt,0) finishes 4 matmuls (~850 ns) before the end:
                # its halved epilogue + both-ring DMAs hide under bank
                # (nt,1)'s matmuls
                for k in range(K_STAG, K_TILES):
                    mm(k, nt, 0, stop=(k == K_TILES - 1))
                orow = out_ext[nt * 128 : (nt + 1) * 128, 0:512]
                i = nt * M_HALVES
                ot0 = epilogue(r, i, nt, 0, 0, 256, on_scalar=True)
                ot1 = epilogue(r, i, nt, 0, 256, 512, on_scalar=False)
                nc.sync.dma_start(orow[:, 0:256], ot0[:, :])
                nc.scalar.dma_start(orow[:, 256:512], ot1[:, :])
                # bank (nt,1) is the ONLY exposed epilogue: quarter it
                # across both engines so the first 32 KB DMA launches
                # ~200 ns after the final matmul and each ring drains two
                # quarters in parallel
                for k in range(K_STAG, K_TILES):
                    mm(k, nt, 1, stop=(k == K_TILES - 1))
                orow = out_ext[nt * 128 : (nt + 1) * 128, 512:1024]
                i = nt * M_HALVES + 1
                for q in range(4):
                    otq = epilogue(
                        r, i, nt, 1, q * 128, (q + 1) * 128,
                        on_scalar=(q % 2 == 0),
                    )
                    eng = nc.sync if q % 2 == 0 else nc.scalar
                    eng.dma_start(orow[:, q * 128 : (q + 1) * 128], otq[:, :])

    nc.compile()
    return nc


def _get_nc():
    if "nc" not in _CACHE:
        _CACHE["nc"] = _build()
    return _CACHE["nc"]


def prep_in_maps(x, values, bias, rows, cols):
    x = np.asarray(x, np.float32)
    values = np.asarray(values, np.float32)
    bias = np.asarray(bias, np.float32)
    rows = np.asarray(rows)
    cols = np.asarray(cols)

    # densify via bincount (vectorized scatter-add; duplicates accumulate)
    flat = rows.astype(np.int64) * UNITS + cols.astype(np.int64)
    W = np.bincount(flat, weights=values.astype(np.float64), minlength=D_IN * UNITS)
    W = W.reshape(D_IN, UNITS).astype(np.float32)

    # partition-major xT: xT_pm[p, k*1024 + m] = x[m, k*128 + p]
    xT16 = np.ascontiguousarray(x.T).astype(ml_dtypes.bfloat16)  # [D_IN, BATCH]
    xT_pm = np.ascontiguousarray(
        xT16.reshape(K_TILES, 128, BATCH).transpose(1, 0, 2).reshape(128, K_TILES * BATCH)
    )
    W16 = W.astype(ml_dtypes.bfloat16)

    in_maps = []
    for i in range(N_CORES):
        w_shard = W16[:, i * N_SHARD : (i + 1) * N_SHARD]  # [D_IN, 512]
        # partition-major W: w_pm[p, k*512 + n] = W[k*128 + p, n0 + n]
        w_pm = np.ascontiguousarray(
            w_shard.reshape(K_TILES, 128, N_SHARD)
            .transpose(1, 0, 2)
            .reshape(128, K_TILES * N_SHARD)
        )
        b_shard = np.ascontiguousarray(
            bias[i * N_SHARD : (i + 1) * N_SHARD].reshape(N_TILES, 128).T
        )
        in_maps.append({"xT": xT_pm, "w": w_pm, "bias": b_shard})
    return in_maps


def kernel(x, values, bias, rows, cols):
    from concourse.bass_utils import run_bass_kernel_spmd

    in_maps = prep_in_maps(x, values, bias, rows, cols)
    nc = _get_nc()
    res = run_bass_kernel_spmd(nc, in_maps, list(range(N_CORES)))
    out = np.empty((BATCH, UNITS), np.float32)
    for i in range(N_CORES):
        out[:, i * N_SHARD : (i + 1) * N_SHARD] = (
            res.results[i]["out"].astype(np.float32).T
        )
    return out



# revision 4
# speedup vs baseline: 1.0536x; 1.0536x over previous
"""Bass/Trainium2 kernel for nn_AdaptiveSparseReservoir (self-contained).

out[b, c] = relu(sum_k x[b, rows[k]] * values[k] for cols[k]==c  + bias[c])
  x [1024, 4096] f32; values [262144] f32; rows/cols [262144] i32;
  bias [4096] f32  ->  out [1024, 4096] f32

Strategy
--------
Densify the sparse COO kernel on the host into W [4096, 4096] (1.6%
density with unstructured support is far too dense for gather/scatter on
TRN2 — a dense bf16 TensorEngine matmul moves ~16x fewer bytes), then run
the dense matmul column-sharded across the 8 NeuronCores with NO
collectives: core i computes outT_i = relu(W[:, 512i:512(i+1)].T @ x.T +
b_i).  PE floor: 256 N=512 bf16 PSUM-accumulating matmuls/core (~203-226
ns each on silicon).  Ruled out: fp8 (DoubleRow only ~1.44x measured and
the accuracy gate needs a 3-term split => slower than bf16), int8 (no
bass matmul dtype), sparsity exploitation (256 nnz per 128x128 tile =>
no block skipping; host pre-gather trades 13% PE for 3.6x DMA).

v4 restructure (vs the v1 staggered-drain kernel; TimelineSim single-shot
67.4us -> 64.2us, silicon rep-slope A/B at R=16/40 chained dispatch:
-4839 ns/body median paired delta, n=41, v4 faster in 28/41):
- PACKED OUTPUT: one persistent SBUF tile [128, 4096] bf16; the DRAM out
  is laid out partition-major [128, nt*1024 + b] so each drain DMA is
  128 descriptors of >=1KB. Host unshuffles (free).
- EARLY BANK DRAIN: phase 2 starts at K_STAG=24; pair nt finishes its
  last 8 k-tiles in a contiguous run, its two epilogues (ACT for mh=0,
  DVE for mh=1) write the packed tile, and ONE 256KB out-DMA per pair
  fires midstream (~10.5/7/3.5 us before stream end) while the input
  rings are idle. Only 5 out-DMAs total (A..E) instead of 12.
- MINIMAL TAIL: the final pair is split bank6-then-bank7; bank6 drains
  1.7 us early (128KB DMA), bank7's epilogue is halved across ACT/DVE
  and followed by the single last 128KB DMA.
- SCHEDULER-PROOF QUEUES: the Tile scheduler orders each engine queue by
  its internal-model readiness, and the issuing sequencers stall FIFO on
  an unsatisfied DMA wait — so any queue mixing input chunks with drains
  can head-of-line block one behind the other (observed: next-body w
  chunks waiting 3.6us on WAR sems AHEAD of the pair-3 epilogues on ACT).
  Assignment here: SP ring = x chunks (k-ordered waits, monotone), ACT
  ring = w chunks, DVE = every epilogue, Pool(SWDGE) = every out-DMA.
  Each queue's readiness order equals its program order by construction.
- REP-BOUNDARY PREFETCH: body r+1's k<24 chunks are emitted before body
  r's final outs; k>=24 chunks (blocked by body r's phase-2 readers
  anyway) go after them.
- MINIMAL FINAL TAIL: the last body's bank7 splits k=31 into two N=256
  halves; the epilogue halves run in parallel on DVE (lo) and ACT (hi,
  act table preloaded midstream) and the two 64KB out-DMAs drain
  concurrently on the Pool and SP queues, both empty at that point.
"""

import types

import numpy as np
import ml_dtypes

D_IN = 4096
UNITS = 4096
NNZ = 262144
BATCH = 1024
N_CORES = 8
N_SHARD = UNITS // N_CORES  # 512 output columns per core
K_TILES = D_IN // 128  # 32
N_TILES = N_SHARD // 128  # 4
M_HALVES = BATCH // 512  # 2
K_STAG = 24  # phase-2 (bank-pair-major) start

_CACHE = {}


def _drain_only(self, tick_clock, wait_clock):
    """Tail = DMA/compute drain only; skip the butterfly barrier + sem
    clears (the Bass preamble sem_clears at the start of each execution,
    and NEFF completion already requires every engine queue to finish)."""
    from concourse.tile import ScopedClock

    drain_inst = self.nc.sync.drain()
    wait_clock.add_sem_waits(
        drain_inst.ins, ScopedClock({None: tick_clock.global_clock})
    )
    popped = self.nc._tile_sem_poison_stack.pop()
    assert popped is self._sem_poison


def _build(reps=1):
    import concourse.mybir as mybir
    import concourse.tile as tile
    from concourse import bacc

    nc = bacc.Bacc("TRN2", target_bir_lowering=False, debug=False, num_devices=N_CORES)
    bf16 = mybir.dt.bfloat16
    f32 = mybir.dt.float32

    xT_ext = nc.declare_dram_parameter("xT", [128, K_TILES * 1024], bf16, isOutput=False)
    w_ext = nc.declare_dram_parameter("w", [128, K_TILES * 512], bf16, isOutput=False)
    b_ext = nc.declare_dram_parameter("bias", [128, N_TILES], f32, isOutput=False)
    out_ext = nc.declare_dram_parameter("out", [128, N_TILES * BATCH], bf16, isOutput=True)

    tc_outer = tile.TileContext(nc)
    try:
        # verify the internals _drain_only touches exist in this concourse
        from concourse.tile import ScopedClock  # noqa: F401

        assert hasattr(tc_outer, "_drain_and_barrier")
        assert hasattr(nc, "_tile_sem_poison_stack")
        tc_outer._drain_and_barrier = types.MethodType(_drain_only, tc_outer)
    except Exception:
        pass  # stock barrier exit: ~4us slower, still correct
    with tc_outer as tc:
        with (
            tc.tile_pool(name="consts", bufs=1) as cpool,
            tc.tile_pool(name="xk", bufs=1) as xpool,
            tc.tile_pool(name="wk", bufs=1) as wpool,
            tc.tile_pool(name="osb", bufs=1) as opool,
            tc.tile_pool(name="psum", bufs=1, space="PSUM") as ppool,
        ):
            psum = [
                ppool.tile([128, 512], f32, tag=f"ps{i}", name=f"ps{i}")
                for i in range(N_TILES * M_HALVES)
            ]

            # PE warm-up against the HAM cold clock: short N=128 matmuls keep
            # the PE-busy window covered until the first data chunk lands.
            # NO memset — the warm-ups read a RAW (untracked, uninitialized)
            # SBUF tensor and start with zero dependencies. Any NaN garbage
            # lands in psum bank 0 whose has_written bits the first real
            # start=True matmul clears (cleared bits force overwrite).
            warm = nc.alloc_sbuf_tensor("warm_raw", [128, 128], bf16)
            for _ in range(5):
                nc.tensor.matmul(
                    psum[0][:, :128], warm[:, 0:128], warm[:, 0:128],
                    start=True, stop=True,
                )

            tbl_warm = cpool.tile([128, 1], f32)
            bias_sb = cpool.tile([128, N_TILES], f32)

            xts = xpool.tile([128, K_TILES * 1024], bf16, name="xts")
            wts = wpool.tile([128, K_TILES * 512], bf16, name="wts")
            osb = opool.tile([128, N_TILES * BATCH], bf16, name="osb")

            # interleave x/w chunks in k order, alternating HWDGE rings;
            # fine-grained first chunks (early PE start). The head is split
            # so the FIRST matmul — an N=256 half of bank (0,0) — waits only
            # on a 64 KB x transfer and a 32 KB w transfer on parallel
            # rings (~0.5 us), instead of two 128 KB ones.
            chunks = [
                ("x", 0, 256), ("w", 0, 128), ("x", 256, 512),
                ("w", 128, 512), ("x", 512, 1024), ("w", 512, 1024),
            ]
            xbounds = [1, 2, 3, 4, 5, 6] + list(range(8, K_TILES + 1, 2))
            wbounds = [2, 3, 4, 6, 8] + list(range(12, K_TILES + 1, 4))
            xi = wi = 0
            while xi < len(xbounds) - 1 or wi < len(wbounds) - 1:
                kx = xbounds[xi] if xi < len(xbounds) - 1 else K_TILES
                kw = wbounds[wi] if wi < len(wbounds) - 1 else K_TILES
                if kw < kx and wi < len(wbounds) - 1:
                    chunks.append(("w", wbounds[wi] * 512, wbounds[wi + 1] * 512))
                    wi += 1
                else:
                    chunks.append(("x", xbounds[xi] * 1024, xbounds[xi + 1] * 1024))
                    xi += 1

            def emit_chunk(i, kind, clo, chi):
                eng = nc.sync if i % 2 == 0 else nc.scalar
                if kind == "x":
                    eng.dma_start(xts[:, clo:chi], xT_ext[:, clo:chi])
                else:
                    eng.dma_start(wts[:, clo:chi], w_ext[:, clo:chi])

            # split: "head" chunks cover k < K_STAG (prefetchable during the
            # PREVIOUS body's phase 2); "rest" chunks cover k >= K_STAG and
            # are blocked by the previous body's phase-2 readers anyway.
            head_chunks, rest_chunks = [], []
            for i, (kind, clo, chi) in enumerate(chunks):
                lim = K_STAG * (1024 if kind == "x" else 512)
                (head_chunks if chi <= lim else rest_chunks).append((i, kind, clo, chi))

            def mm(k, nt, mh, stop=False):
                nc.tensor.matmul(
                    psum[nt * M_HALVES + mh][:, :],
                    wts[:, k * 512 + nt * 128 : k * 512 + (nt + 1) * 128],
                    xts[:, k * 1024 + mh * 512 : k * 1024 + (mh + 1) * 512],
                    start=(k == 0),
                    stop=stop,
                )

            def epilogue(nt, mh, lo, hi):
                # bias+relu from psum bank (nt,mh) into the packed out tile.
                # ALL epilogues run on DVE: the DVE queue carries nothing
                # else, so its FIFO order always matches readiness order and
                # an epilogue can never sit behind a DMA waiting on a
                # semaphore (ACT/SP queues carry the input streams and would
                # head-of-line-block drains behind next-body chunk WARs).
                dst = osb[:, nt * 1024 + mh * 512 + lo : nt * 1024 + mh * 512 + hi]
                nc.vector.tensor_scalar(
                    dst,
                    psum[nt * M_HALVES + mh][:, lo:hi],
                    bias_sb[:, nt : nt + 1],
                    0.0,
                    mybir.AluOpType.add,
                    mybir.AluOpType.max,
                )

            def emit_out(c0, c1):
                # out-DMAs ride the Pool/GPSIMD SWDGE queue: it is otherwise
                # idle, so drains never contend with (or block) the input
                # rings; the ~1us SWDGE setup hides under the matmul stream.
                nc.gpsimd.dma_start(out_ext[:, c0:c1], osb[:, c0:c1])

            for r in range(reps):
                if r == 0:
                    for i, kind, clo, chi in head_chunks:
                        emit_chunk(i, kind, clo, chi)
                    for i, kind, clo, chi in rest_chunks:
                        emit_chunk(i, kind, clo, chi)
                    # bias (2 KB) rides behind the input stream — needed
                    # only by the first epilogue ~14 us before stream end
                    nc.sync.dma_start(bias_sb[:, :], b_ext[:, :])
                    # trigger the Relu act-table load now (ACT is idle during
                    # the stream); bacc hoists LoadActFuncSet before this
                    # instruction, keeping the ~1.3us load off the final
                    # epilogue-half critical path
                    nc.scalar.activation(
                        tbl_warm[:, :], warm[:, 0:1],
                        mybir.ActivationFunctionType.Relu,
                    )

                # k=0, mh=0 first across all nt (only x cols 0:512 needed);
                # bank (0,0) starts as two N=256 halves gated on just the
                # first 64 KB x chunk. start=True clears has_written for the
                # WHOLE bank, so only the first half sets it; the second
                # half runs start=False and overwrites its cleared region.
                nc.tensor.matmul(
                    psum[0][:, 0:256], wts[:, 0:128], xts[:, 0:256],
                    start=True, stop=False,
                )
                nc.tensor.matmul(
                    psum[0][:, 256:512], wts[:, 0:128], xts[:, 256:512],
                    start=False, stop=False,
                )
                for nt in range(1, N_TILES):
                    mm(0, nt, 0)
                for nt in range(N_TILES):
                    mm(0, nt, 1)

                # phase 1: k-major over all banks — keeps the PE stream dense
                # while DMA feeds k-tiles. mh-inner so each LDWEIGHTS serves
                # two matmuls.
                for k in range(1, K_STAG):
                    for nt in range(N_TILES):
                        for mh in range(M_HALVES):
                            mm(k, nt, mh)

                if r < reps - 1:
                    # rep-boundary prefetch BEFORE this body's outs hit the
                    # rings: the sequencers process DMAs in FIFO order and an
                    # out-DMA waiting on a late epilogue would block the next
                    # body's head behind it. These chunks' WAR deps (this
                    # body's phase-1 reads of k<K_STAG) clear midstream, so
                    # they land long before the boundary.
                    for i, kind, clo, chi in head_chunks:
                        emit_chunk(i, kind, clo, chi)

                # phase 2: bank-pair-major — pair nt runs its last 8 k-tiles
                # (mh pairs share LDWEIGHTS), drains both banks (fused
                # bias+relu, ACT for mh=0 / DVE for mh=1) into the packed
                # tile, and fires ONE 256KB out-DMA. Pair completions are
                # ~3.4 us apart, so every drain pipelines under the
                # remaining matmul stream with the input rings idle.
                for nt in range(N_TILES - 1):
                    for k in range(K_STAG, K_TILES):
                        mm(k, nt, 0, stop=(k == K_TILES - 1))
                        mm(k, nt, 1, stop=(k == K_TILES - 1))
                    epilogue(nt, 0, 0, 512)
                    epilogue(nt, 1, 0, 512)
                    emit_out(nt * 1024, (nt + 1) * 1024)

                # final pair (nt=3): bank6 drains 8 matmuls (~1.7us) early;
                # bank7's epilogue + 128KB DMA are the only exposed tail.
                nt = N_TILES - 1
                for k in range(K_STAG, K_TILES):
                    mm(k, nt, 0, stop=(k == K_TILES - 1))
                epilogue(nt, 0, 0, 512)
                emit_out(nt * 1024, nt * 1024 + 512)  # D: bank6
                if r < reps - 1:
                    for k in range(K_STAG, K_TILES):
                        mm(k, nt, 1, stop=(k == K_TILES - 1))
                    epilogue(nt, 1, 0, 512)
                    emit_out(nt * 1024 + 512, (nt + 1) * 1024)  # E: bank7
                else:
                    # last body: minimal exposed tail. k=31 is split into two
                    # N=256 halves so the lo half of the bank retires ~107ns
                    # early; the epilogue halves run in PARALLEL on DVE (lo)
                    # and ACT (hi, fused bias+relu via the preloaded act
                    # table); the two 64KB out-DMAs drain concurrently on the
                    # Pool(SWDGE) and SP(HWDGE) queues. Both queues are
                    # empty at this point in the final body, so nothing can
                    # head-of-line block them.
                    for k in range(K_STAG, K_TILES - 1):
                        mm(k, nt, 1, stop=False)
                    c7 = nt * 1024 + 512
                    nc.tensor.matmul(
                        psum[nt * M_HALVES + 1][:, 0:256],
                        wts[:, 31 * 512 + nt * 128 : 31 * 512 + (nt + 1) * 128],
                        xts[:, 31 * 1024 + 512 : 31 * 1024 + 768],
                        start=False, stop=False,
                    )
                    nc.tensor.matmul(
                        psum[nt * M_HALVES + 1][:, 256:512],
                        wts[:, 31 * 512 + nt * 128 : 31 * 512 + (nt + 1) * 128],
                        xts[:, 31 * 1024 + 768 : 31 * 1024 + 1024],
                        start=False, stop=True,
                    )
                    epilogue(nt, 1, 0, 256)  # lo on DVE
                    nc.scalar.activation(
                        osb[:, c7 + 256 : c7 + 512],
                        psum[nt * M_HALVES + 1][:, 256:512],
                        mybir.ActivationFunctionType.Relu,
                        bias=bias_sb[:, nt : nt + 1],
                    )
                    nc.gpsimd.dma_start(
                        out_ext[:, c7 : c7 + 256], osb[:, c7 : c7 + 256]
                    )
                    nc.sync.dma_start(
                        out_ext[:, c7 + 256 : c7 + 512], osb[:, c7 + 256 : c7 + 512]
                    )
                if r < reps - 1:
                    # k>=K_STAG chunks are blocked by this body's phase-2
                    # readers anyway; they go behind the final outs.
                    for i, kind, clo, chi in rest_chunks:
                        emit_chunk(i, kind, clo, chi)

    nc.compile()
    return nc


def _get_nc():
    if "nc" not in _CACHE:
        _CACHE["nc"] = _build()
    return _CACHE["nc"]


def prep_in_maps(x, values, bias, rows, cols):
    x = np.asarray(x, np.float32)
    values = np.asarray(values, np.float32)
    bias = np.asarray(bias, np.float32)
    rows = np.asarray(rows)
    cols = np.asarray(cols)

    # densify via bincount (vectorized scatter-add; duplicates accumulate)
    flat = rows.astype(np.int64) * UNITS + cols.astype(np.int64)
    W = np.bincount(flat, weights=values.astype(np.float64), minlength=D_IN * UNITS)
    W = W.reshape(D_IN, UNITS).astype(np.float32)

    # partition-major xT: xT_pm[p, k*1024 + m] = x[m, k*128 + p]
    xT16 = np.ascontiguousarray(x.T).astype(ml_dtypes.bfloat16)  # [D_IN, BATCH]
    xT_pm = np.ascontiguousarray(
        xT16.reshape(K_TILES, 128, BATCH).transpose(1, 0, 2).reshape(128, K_TILES * BATCH)
    )
    W16 = W.astype(ml_dtypes.bfloat16)

    in_maps = []
    for i in range(N_CORES):
        w_shard = W16[:, i * N_SHARD : (i + 1) * N_SHARD]  # [D_IN, 512]
        # partition-major W: w_pm[p, k*512 + n] = W[k*128 + p, n0 + n]
        w_pm = np.ascontiguousarray(
            w_shard.reshape(K_TILES, 128, N_SHARD)
            .transpose(1, 0, 2)
            .reshape(128, K_TILES * N_SHARD)
        )
        b_shard = np.ascontiguousarray(
            bias[i * N_SHARD : (i + 1) * N_SHARD].reshape(N_TILES, 128).T
        )
        in_maps.append({"xT": xT_pm, "w": w_pm, "bias": b_shard})
    return in_maps


def kernel(x, values, bias, rows, cols):
    from concourse.bass_utils import run_bass_kernel_spmd

    in_maps = prep_in_maps(x, values, bias, rows, cols)
    nc = _get_nc()
    res = run_bass_kernel_spmd(nc, in_maps, list(range(N_CORES)))
    out = np.empty((BATCH, UNITS), np.float32)
    for i in range(N_CORES):
        # packed [128, nt*1024 + b] -> out[b, 512*i + nt*128 + p]
        arr = res.results[i]["out"].astype(np.float32).reshape(128, N_TILES, BATCH)
        out[:, i * N_SHARD : (i + 1) * N_SHARD] = (
            arr.transpose(1, 0, 2).reshape(N_SHARD, BATCH).T
        )
    return out


# revision 5
# speedup vs baseline: 1.1569x; 1.0980x over previous
"""Bass/Trainium2 kernel for nn_AdaptiveSparseReservoir (self-contained).

out[b, c] = relu(sum_k x[b, rows[k]] * values[k] for cols[k]==c  + bias[c])
  x [1024, 4096] f32; values [262144] f32; rows/cols [262144] i32;
  bias [4096] f32  ->  out [1024, 4096] f32

Strategy
--------
Densify the sparse COO kernel on the host into W [4096, 4096] (1.6%
density with unstructured support is far too dense for gather/scatter on
TRN2), then run the dense matmul column-sharded across the 8 NeuronCores
with NO collectives: core i computes outT_i = relu(W[:, 512i:512(i+1)].T
@ x.T + b_i).  PE floor: 256 N=512 bf16 matmuls/core ~= 55.8 us.

v6 = v4 drain restructure + PARTIAL FP8: k-tiles 0..3 (1/8 of K) run as
fp8e4m3 DoubleRow passes (2 k-tiles per pass, both operands e4m3, 3D
[128,2,n] APs, out free = rhs.free/2) INSIDE phase 2, so the bf16
head/early-start path is untouched and the fp8 operands stream in behind
the bf16 stream. Replaces 4 bf16 k-passes (2048 cycles/bank) with 2
DoubleRow passes (~1160): ~2-3 us/body less PE stream (TimelineSim
slope 54.5 -> 49.4 us/body). Quantization cost measured EXACTLY against
the reference (deterministic seed): L2 2.87e-3 (bf16) -> 1.359e-2 on
silicon (numpy predicted 1.349e-2), 47% headroom under the 2e-2 gate.
All-fp8 (3.7%) and f=1/4 (1.87e-2) rejected for margin; fp8 fractions
beyond 1/8 scale noise as ~3.7%*sqrt(f).

v4 drain restructure (vs the staggered-drain v1):
- PACKED OUTPUT: one persistent SBUF tile [128, 4096] bf16; the DRAM out
  is laid out partition-major [128, nt*1024 + b] so each drain DMA is
  128 descriptors of >=1KB. Host unshuffles (free).
- EARLY BANK DRAIN: phase 2 starts at K_STAG=24; pair nt finishes its
  last 8 k-tiles in a contiguous run, its two epilogues (ACT for mh=0,
  DVE for mh=1) write the packed tile, and ONE 256KB out-DMA per pair
  fires midstream (~10.5/7/3.5 us before stream end) while the input
  rings are idle. Only 5 out-DMAs total (A..E) instead of 12.
- MINIMAL TAIL: the final pair is split bank6-then-bank7; bank6 drains
  1.7 us early (128KB DMA), bank7's epilogue is halved across ACT/DVE
  and followed by the single last 128KB DMA.
- REP-BOUNDARY PREFETCH: body r+1's input chunks for k<24 are emitted
  BEFORE body r's final outs (D,E) on the rings, so the next body's head
  never queues behind a drain that only completes at stream end; chunks
  k>=24 (blocked by body r's phase-2 readers anyway) go after.
"""

import types

import numpy as np
import ml_dtypes

D_IN = 4096
UNITS = 4096
NNZ = 262144
BATCH = 1024
N_CORES = 8
N_SHARD = UNITS // N_CORES  # 512 output columns per core
K_TILES = D_IN // 128  # 32
N_TILES = N_SHARD // 128  # 4
M_HALVES = BATCH // 512  # 2
K_STAG = 24  # phase-2 (bank-pair-major) start
# k-tiles 0..K_FP8-1 run as fp8e4m3 DoubleRow matmuls (2 k-tiles per pass at
# ~1.13x a bf16 pass => ~1.77x per k-row): cuts ~3 us of PE stream per body.
# Quantization noise is ~3.7% for ALL-fp8 (measured against the exact
# reference on this data); at f=4/32 of K it is sqrt(f)-scaled: measured
# L2 = 1.35e-2 vs the 2e-2 gate (48% headroom). The fp8 passes run inside
# phase 2 (accumulation order is free), so the bf16 head/early-start path
# is untouched and the fp8 operands stream in BEHIND the bf16 stream.
K_FP8 = 4

_CACHE = {}


def _drain_only(self, tick_clock, wait_clock):
    """Tail = DMA/compute drain only; skip the butterfly barrier + sem
    clears (the Bass preamble sem_clears at the start of each execution,
    and NEFF completion already requires every engine queue to finish)."""
    from concourse.tile import ScopedClock

    drain_inst = self.nc.sync.drain()
    wait_clock.add_sem_waits(
        drain_inst.ins, ScopedClock({None: tick_clock.global_clock})
    )
    popped = self.nc._tile_sem_poison_stack.pop()
    assert popped is self._sem_poison


def _build(reps=1):
    import concourse.mybir as mybir
    import concourse.tile as tile
    from concourse import bacc

    nc = bacc.Bacc("TRN2", target_bir_lowering=False, debug=False, num_devices=N_CORES)
    bf16 = mybir.dt.bfloat16
    f32 = mybir.dt.float32

    f8 = mybir.dt.float8e4
    xT_ext = nc.declare_dram_parameter("xT", [128, K_TILES * 1024], bf16, isOutput=False)
    w_ext = nc.declare_dram_parameter("w", [128, K_TILES * 512], bf16, isOutput=False)
    x8_ext = nc.declare_dram_parameter("x8", [128, K_FP8 * 1024], f8, isOutput=False)
    w8_ext = nc.declare_dram_parameter("w8", [128, K_FP8 * 512], f8, isOutput=False)
    b_ext = nc.declare_dram_parameter("bias", [128, N_TILES], f32, isOutput=False)
    out_ext = nc.declare_dram_parameter("out", [128, N_TILES * BATCH], bf16, isOutput=True)

    tc_outer = tile.TileContext(nc)
    try:
        # verify the internals _drain_only touches exist in this concourse
        from concourse.tile import ScopedClock  # noqa: F401

        assert hasattr(tc_outer, "_drain_and_barrier")
        assert hasattr(nc, "_tile_sem_poison_stack")
        tc_outer._drain_and_barrier = types.MethodType(_drain_only, tc_outer)
    except Exception:
        pass  # stock barrier exit: ~4us slower, still correct
    with tc_outer as tc:
        with (
            tc.tile_pool(name="consts", bufs=1) as cpool,
            tc.tile_pool(name="xk", bufs=1) as xpool,
            tc.tile_pool(name="wk", bufs=1) as wpool,
            tc.tile_pool(name="osb", bufs=1) as opool,
            tc.tile_pool(name="psum", bufs=1, space="PSUM") as ppool,
        ):
            psum = [
                ppool.tile([128, 512], f32, tag=f"ps{i}", name=f"ps{i}")
                for i in range(N_TILES * M_HALVES)
            ]

            # PE warm-up against the HAM cold clock: short N=128 matmuls keep
            # the PE-busy window covered until the first data chunk lands.
            # NO memset — the warm-ups read a RAW (untracked, uninitialized)
            # SBUF tensor and start with zero dependencies. Any NaN garbage
            # lands in psum bank 0 whose has_written bits the first real
            # start=True matmul clears (cleared bits force overwrite).
            warm = nc.alloc_sbuf_tensor("warm_raw", [128, 128], bf16)
            for _ in range(5):
                nc.tensor.matmul(
                    psum[0][:, :128], warm[:, 0:128], warm[:, 0:128],
                    start=True, stop=True,
                )

            tbl_warm = cpool.tile([128, 1], f32)
            bias_sb = cpool.tile([128, N_TILES], f32)

            xts = xpool.tile([128, K_TILES * 1024], bf16, name="xts")
            wts = wpool.tile([128, K_TILES * 512], bf16, name="wts")
            x8 = xpool.tile([128, K_FP8, 1024], f8, name="x8")
            w8 = wpool.tile([128, K_FP8, 512], f8, name="w8")
            osb = opool.tile([128, N_TILES * BATCH], bf16, name="osb")

            # interleave x/w chunks in k order, alternating HWDGE rings;
            # fine-grained first chunks (early PE start). The head is split
            # so the FIRST matmul — an N=256 half of bank (0,0) — waits only
            # on a 64 KB x transfer and a 32 KB w transfer on parallel
            # rings (~0.5 us), instead of two 128 KB ones.
            F = K_FP8  # bf16 stream starts at k-tile K_FP8
            chunks = [
                ("x", F * 1024, F * 1024 + 256), ("w", F * 512, F * 512 + 128),
                ("x", F * 1024 + 256, F * 1024 + 512),
                ("w", F * 512 + 128, (F + 1) * 512),
                ("x", F * 1024 + 512, (F + 1) * 1024),
                ("w", (F + 1) * 512, (F + 2) * 512),
            ]
            xbounds = [F + 1, F + 2, F + 3, F + 4, F + 5, F + 6] + list(
                range(F + 8, K_TILES + 1, 2)
            )
            wbounds = [F + 2, F + 3, F + 4, F + 6, F + 8] + list(
                range(F + 12, K_TILES + 1, 4)
            )
            xi = wi = 0
            while xi < len(xbounds) - 1 or wi < len(wbounds) - 1:
                kx = xbounds[xi] if xi < len(xbounds) - 1 else K_TILES
                kw = wbounds[wi] if wi < len(wbounds) - 1 else K_TILES
                if kw < kx and wi < len(wbounds) - 1:
                    chunks.append(("w", wbounds[wi] * 512, wbounds[wi + 1] * 512))
                    wi += 1
                else:
                    chunks.append(("x", xbounds[xi] * 1024, xbounds[xi + 1] * 1024))
                    xi += 1
            # fp8 operands (k-tiles 0..K_FP8-1, bounds in k-subtile units):
            # consumed by phase 2, streamed behind the bf16 stream
            fp8_chunks = [("x8", 0, 2), ("w8", 0, K_FP8), ("x8", 2, K_FP8)]

            def emit_chunk(i, kind, clo, chi):
                eng = nc.sync if i % 2 == 0 else nc.scalar
                if kind == "x":
                    eng.dma_start(xts[:, clo:chi], xT_ext[:, clo:chi])
                elif kind == "w":
                    eng.dma_start(wts[:, clo:chi], w_ext[:, clo:chi])
                elif kind == "x8":
                    eng.dma_start(x8[:, clo:chi, :], x8_ext[:, clo * 1024 : chi * 1024])
                else:
                    eng.dma_start(w8[:, clo:chi, :], w8_ext[:, clo * 512 : chi * 512])

            # split: "head" chunks cover k < K_STAG (prefetchable during the
            # PREVIOUS body's phase 2); "rest" chunks (bf16 k >= K_STAG and
            # all fp8 — both read by the previous body's phase 2) go after
            # the final outs. fp8 first: their WAR clears ~4 us earlier.
            head_chunks, rest_chunks = [], []
            for i, (kind, clo, chi) in enumerate(chunks):
                lim = K_STAG * (1024 if kind == "x" else 512)
                (head_chunks if chi <= lim else rest_chunks).append((i, kind, clo, chi))
            rest_chunks = [
                (len(chunks) + j, kind, clo, chi)
                for j, (kind, clo, chi) in enumerate(fp8_chunks)
            ] + rest_chunks

            def mm(k, nt, mh, stop=False):
                nc.tensor.matmul(
                    psum[nt * M_HALVES + mh][:, :],
                    wts[:, k * 512 + nt * 128 : k * 512 + (nt + 1) * 128],
                    xts[:, k * 1024 + mh * 512 : k * 1024 + (mh + 1) * 512],
                    start=(k == K_FP8),
                    stop=stop,
                )

            def mm8(kp, nt, mh):
                # fp8e4m3 DoubleRow: one pass covers k-tiles kp, kp+1 (the
                # 3D [128, 2, n] APs pair k-subtiles per PE cell); accumulates
                # into the bank mid-group (start/stop live on the bf16 ends)
                nc.tensor.matmul(
                    psum[nt * M_HALVES + mh][:, :],
                    w8[:, kp : kp + 2, nt * 128 : (nt + 1) * 128],
                    x8[:, kp : kp + 2, mh * 512 : (mh + 1) * 512],
                    start=False,
                    stop=False,
                    perf_mode=mybir.MatmulPerfMode.DoubleRow,
                )

            def epilogue(nt, mh, lo, hi):
                # bias+relu from psum bank (nt,mh) into the packed out tile.
                # ALL epilogues run on DVE: the DVE queue carries nothing
                # else, so its FIFO order always matches readiness order and
                # an epilogue can never sit behind a DMA waiting on a
                # semaphore (ACT/SP queues carry the input streams and would
                # head-of-line-block drains behind next-body chunk WARs).
                dst = osb[:, nt * 1024 + mh * 512 + lo : nt * 1024 + mh * 512 + hi]
                nc.vector.tensor_scalar(
                    dst,
                    psum[nt * M_HALVES + mh][:, lo:hi],
                    bias_sb[:, nt : nt + 1],
                    0.0,
                    mybir.AluOpType.add,
                    mybir.AluOpType.max,
                )

            def emit_out(c0, c1):
                # out-DMAs ride the Pool/GPSIMD SWDGE queue: it is otherwise
                # idle, so drains never contend with (or block) the input
                # rings; the ~1us SWDGE setup hides under the matmul stream.
                nc.gpsimd.dma_start(out_ext[:, c0:c1], osb[:, c0:c1])

            for r in range(reps):
                if r == 0:
                    for i, kind, clo, chi in head_chunks:
                        emit_chunk(i, kind, clo, chi)
                    for i, kind, clo, chi in rest_chunks:
                        emit_chunk(i, kind, clo, chi)
                    # bias (2 KB) rides behind the input stream — needed
                    # only by the first epilogue ~14 us before stream end
                    nc.sync.dma_start(bias_sb[:, :], b_ext[:, :])
                    # trigger the Relu act-table load now (ACT is idle during
                    # the stream); bacc hoists LoadActFuncSet before this
                    # instruction, keeping the ~1.3us load off the final
                    # epilogue-half critical path
                    nc.scalar.activation(
                        tbl_warm[:, :], warm[:, 0:1],
                        mybir.ActivationFunctionType.Relu,
                    )

                # k=K_FP8 (first bf16 k-tile), mh=0 first across all nt;
                # bank (0,0) starts as two N=256 halves gated on just the
                # first 64 KB x chunk. start=True clears has_written for the
                # WHOLE bank, so only the first half sets it; the second
                # half runs start=False and overwrites its cleared region.
                kf = K_FP8
                nc.tensor.matmul(
                    psum[0][:, 0:256],
                    wts[:, kf * 512 : kf * 512 + 128],
                    xts[:, kf * 1024 : kf * 1024 + 256],
                    start=True, stop=False,
                )
                nc.tensor.matmul(
                    psum[0][:, 256:512],
                    wts[:, kf * 512 : kf * 512 + 128],
                    xts[:, kf * 1024 + 256 : kf * 1024 + 512],
                    start=False, stop=False,
                )
                for nt in range(1, N_TILES):
                    mm(kf, nt, 0)
                for nt in range(N_TILES):
                    mm(kf, nt, 1)

                # phase 1: k-major over all banks — keeps the PE stream dense
                # while DMA feeds k-tiles. mh-inner so each LDWEIGHTS serves
                # two matmuls.
                for k in range(K_FP8 + 1, K_STAG):
                    for nt in range(N_TILES):
                        for mh in range(M_HALVES):
                            mm(k, nt, mh)

                if r < reps - 1:
                    # rep-boundary prefetch BEFORE this body's outs hit the
                    # rings: the sequencers process DMAs in FIFO order and an
                    # out-DMA waiting on a late epilogue would block the next
                    # body's head behind it. These chunks' WAR deps (this
                    # body's phase-1 reads of k<K_STAG) clear midstream, so
                    # they land long before the boundary.
                    for i, kind, clo, chi in head_chunks:
                        emit_chunk(i, kind, clo, chi)

                # phase 2: bank-pair-major — pair nt runs its last 8 k-tiles
                # (mh pairs share LDWEIGHTS), drains both banks (fused
                # bias+relu, ACT for mh=0 / DVE for mh=1) into the packed
                # tile, and fires ONE 256KB out-DMA. Pair completions are
                # ~3.4 us apart, so every drain pipelines under the
                # remaining matmul stream with the input rings idle.
                for nt in range(N_TILES - 1):
                    for kp in range(0, K_FP8, 2):
                        mm8(kp, nt, 0)
                        mm8(kp, nt, 1)
                    for k in range(K_STAG, K_TILES):
                        mm(k, nt, 0, stop=(k == K_TILES - 1))
                        mm(k, nt, 1, stop=(k == K_TILES - 1))
                    epilogue(nt, 0, 0, 512)
                    epilogue(nt, 1, 0, 512)
                    emit_out(nt * 1024, (nt + 1) * 1024)

                # final pair (nt=3): bank6 drains 8 matmuls (~1.7us) early;
                # bank7's epilogue + 128KB DMA are the only exposed tail.
                nt = N_TILES - 1
                for kp in range(0, K_FP8, 2):
                    mm8(kp, nt, 0)
                for k in range(K_STAG, K_TILES):
                    mm(k, nt, 0, stop=(k == K_TILES - 1))
                epilogue(nt, 0, 0, 512)
                emit_out(nt * 1024, nt * 1024 + 512)  # D: bank6
                for kp in range(0, K_FP8, 2):
                    mm8(kp, nt, 1)
                if r < reps - 1:
                    for k in range(K_STAG, K_TILES):
                        mm(k, nt, 1, stop=(k == K_TILES - 1))
                    epilogue(nt, 1, 0, 512)
                    emit_out(nt * 1024 + 512, (nt + 1) * 1024)  # E: bank7
                else:
                    # last body: minimal exposed tail. k=31 is split into two
                    # N=256 halves so the lo half of the bank retires ~107ns
                    # early; the epilogue halves run in PARALLEL on DVE (lo)
                    # and ACT (hi, fused bias+relu via the preloaded act
                    # table); the two 64KB out-DMAs drain concurrently on the
                    # Pool(SWDGE) and SP(HWDGE) queues. Both queues are
                    # empty at this point in the final body, so nothing can
                    # head-of-line block them.
                    for k in range(K_STAG, K_TILES - 1):
                        mm(k, nt, 1, stop=False)
                    c7 = nt * 1024 + 512
                    nc.tensor.matmul(
                        psum[nt * M_HALVES + 1][:, 0:256],
                        wts[:, 31 * 512 + nt * 128 : 31 * 512 + (nt + 1) * 128],
                        xts[:, 31 * 1024 + 512 : 31 * 1024 + 768],
                        start=False, stop=False,
                    )
                    nc.tensor.matmul(
                        psum[nt * M_HALVES + 1][:, 256:512],
                        wts[:, 31 * 512 + nt * 128 : 31 * 512 + (nt + 1) * 128],
                        xts[:, 31 * 1024 + 768 : 31 * 1024 + 1024],
                        start=False, stop=True,
                    )
                    epilogue(nt, 1, 0, 256)  # lo on DVE
                    nc.scalar.activation(
                        osb[:, c7 + 256 : c7 + 512],
                        psum[nt * M_HALVES + 1][:, 256:512],
                        mybir.ActivationFunctionType.Relu,
                        bias=bias_sb[:, nt : nt + 1],
                    )
                    nc.gpsimd.dma_start(
                        out_ext[:, c7 : c7 + 256], osb[:, c7 : c7 + 256]
                    )
                    nc.sync.dma_start(
                        out_ext[:, c7 + 256 : c7 + 512], osb[:, c7 + 256 : c7 + 512]
                    )
                if r < reps - 1:
                    # k>=K_STAG chunks are blocked by this body's phase-2
                    # readers anyway; they go behind the final outs.
                    for i, kind, clo, chi in rest_chunks:
                        emit_chunk(i, kind, clo, chi)

    nc.compile()
    return nc


def _get_nc():
    if "nc" not in _CACHE:
        _CACHE["nc"] = _build()
    return _CACHE["nc"]


def prep_in_maps(x, values, bias, rows, cols):
    x = np.asarray(x, np.float32)
    values = np.asarray(values, np.float32)
    bias = np.asarray(bias, np.float32)
    rows = np.asarray(rows)
    cols = np.asarray(cols)

    # densify via bincount (vectorized scatter-add; duplicates accumulate)
    flat = rows.astype(np.int64) * UNITS + cols.astype(np.int64)
    W = np.bincount(flat, weights=values.astype(np.float64), minlength=D_IN * UNITS)
    W = W.reshape(D_IN, UNITS).astype(np.float32)

    # partition-major xT: xT_pm[p, k*1024 + m] = x[m, k*128 + p]
    xT32 = np.ascontiguousarray(x.T)  # [D_IN, BATCH]
    xT16 = xT32.astype(ml_dtypes.bfloat16)
    xT_pm = np.ascontiguousarray(
        xT16.reshape(K_TILES, 128, BATCH).transpose(1, 0, 2).reshape(128, K_TILES * BATCH)
    )
    # fp8 part: k-tiles 0..K_FP8-1 (values well inside e4m3 range, no clip)
    x8 = xT32[: K_FP8 * 128].astype(ml_dtypes.float8_e4m3)
    x8_pm = np.ascontiguousarray(
        x8.reshape(K_FP8, 128, BATCH).transpose(1, 0, 2).reshape(128, K_FP8 * BATCH)
    )
    W16 = W.astype(ml_dtypes.bfloat16)
    W8 = W[: K_FP8 * 128].astype(ml_dtypes.float8_e4m3)

    in_maps = []
    for i in range(N_CORES):
        w_shard = W16[:, i * N_SHARD : (i + 1) * N_SHARD]  # [D_IN, 512]
        # partition-major W: w_pm[p, k*512 + n] = W[k*128 + p, n0 + n]
        w_pm = np.ascontiguousarray(
            w_shard.reshape(K_TILES, 128, N_SHARD)
            .transpose(1, 0, 2)
            .reshape(128, K_TILES * N_SHARD)
        )
        w8_shard = W8[:, i * N_SHARD : (i + 1) * N_SHARD]  # [K_FP8*128, 512]
        w8_pm = np.ascontiguousarray(
            w8_shard.reshape(K_FP8, 128, N_SHARD)
            .transpose(1, 0, 2)
            .reshape(128, K_FP8 * N_SHARD)
        )
        b_shard = np.ascontiguousarray(
            bias[i * N_SHARD : (i + 1) * N_SHARD].reshape(N_TILES, 128).T
        )
        in_maps.append(
            {"xT": xT_pm, "w": w_pm, "x8": x8_pm, "w8": w8_pm, "bias": b_shard}
        )
    return in_maps


def kernel(x, values, bias, rows, cols):
    from concourse.bass_utils import run_bass_kernel_spmd

    in_maps = prep_in_maps(x, values, bias, rows, cols)
    nc = _get_nc()
    res = run_bass_kernel_spmd(nc, in_maps, list(range(N_CORES)))
    out = np.empty((BATCH, UNITS), np.float32)
    for i in range(N_CORES):
        # packed [128, nt*1024 + b] -> out[b, 512*i + nt*128 + p]
        arr = res.results[i]["out"].astype(np.float32).reshape(128, N_TILES, BATCH)
        out[:, i * N_SHARD : (i + 1) * N_SHARD] = (
            arr.transpose(1, 0, 2).reshape(N_SHARD, BATCH).T
        )
    return out


# revision 6
# speedup vs baseline: 1.1919x; 1.0303x over previous
"""Bass/Trainium2 kernel for nn_AdaptiveSparseReservoir (self-contained).

out[b, c] = relu(sum_k x[b, rows[k]] * values[k] for cols[k]==c  + bias[c])
  x [1024, 4096] f32; values [262144] f32; rows/cols [262144] i32;
  bias [4096] f32  ->  out [1024, 4096] f32

Strategy
--------
Densify the sparse COO kernel on the host into W [4096, 4096] (1.6%
density with unstructured support is far too dense for gather/scatter on
TRN2), then run the dense matmul column-sharded across the 8 NeuronCores
with NO collectives: core i computes outT_i = relu(W[:, 512i:512(i+1)].T
@ x.T + b_i).  PE floor: 256 N=512 bf16 matmuls/core ~= 55.8 us.

v7 = v4 drain restructure + PARTIAL FP8: k-tiles 0..5 (3/16 of K) run as
fp8e4m3 DoubleRow passes (2 k-tiles per pass, both operands e4m3, 3D
[128,2,n] APs, out free = rhs.free/2) INSIDE phase 2, so the bf16
head/early-start path is untouched and the fp8 operands stream in behind
the bf16 stream. Replaces 6 bf16 k-passes (3072 cycles/bank) with 3
DoubleRow passes (~1740): ~4 us/body less PE stream than all-bf16
(TimelineSim slope 54.5 -> 46.9 us/body). Quantization cost is
deterministic (fixed seed) and measured EXACTLY against the reference:
L2 2.87e-3 (bf16) -> 1.642e-2 on silicon (numpy predicted 1.634e-2),
22% headroom under the 2e-2 gate. fp8 noise scales as ~3.7%*sqrt(f):
f=1/8 = 1.359e-2 (prior checkpoint), f=1/4 = 1.87e-2 (rejected, 6.7%
margin), all-fp8 = 3.7%. DoubleRow at 2 fp8 k-rows/cell costs ~1.13x a
bf16 pass (1.77x per k-row), so splitting terms for error correction
can never beat just picking f — only the fraction is tunable.

v4 drain restructure (vs the staggered-drain v1):
- PACKED OUTPUT: one persistent SBUF tile [128, 4096] bf16; the DRAM out
  is laid out partition-major [128, nt*1024 + b] so each drain DMA is
  128 descriptors of >=1KB. Host unshuffles (free).
- EARLY BANK DRAIN: phase 2 starts at K_STAG=24; pair nt finishes its
  last 8 k-tiles in a contiguous run, its two epilogues (ACT for mh=0,
  DVE for mh=1) write the packed tile, and ONE 256KB out-DMA per pair
  fires midstream (~10.5/7/3.5 us before stream end) while the input
  rings are idle. Only 5 out-DMAs total (A..E) instead of 12.
- MINIMAL TAIL: the final pair is split bank6-then-bank7; bank6 drains
  1.7 us early (128KB DMA), bank7's epilogue is halved across ACT/DVE
  and followed by the single last 128KB DMA.
- REP-BOUNDARY PREFETCH: body r+1's input chunks for k<24 are emitted
  BEFORE body r's final outs (D,E) on the rings, so the next body's head
  never queues behind a drain that only completes at stream end; chunks
  k>=24 (blocked by body r's phase-2 readers anyway) go after.
"""

import types

import numpy as np
import ml_dtypes

D_IN = 4096
UNITS = 4096
NNZ = 262144
BATCH = 1024
N_CORES = 8
N_SHARD = UNITS // N_CORES  # 512 output columns per core
K_TILES = D_IN // 128  # 32
N_TILES = N_SHARD // 128  # 4
M_HALVES = BATCH // 512  # 2
K_STAG = 24  # phase-2 (bank-pair-major) start
# k-tiles 0..K_FP8-1 run as fp8e4m3 DoubleRow matmuls (2 k-tiles per pass at
# ~1.13x a bf16 pass => ~1.77x per k-row): cuts ~3 us of PE stream per body.
# Quantization noise is ~3.7% for ALL-fp8 (measured against the exact
# reference on this data); at f=4/32 of K it is sqrt(f)-scaled: measured
# L2 = 1.35e-2 vs the 2e-2 gate (48% headroom). The fp8 passes run inside
# phase 2 (accumulation order is free), so the bf16 head/early-start path
# is untouched and the fp8 operands stream in BEHIND the bf16 stream.
K_FP8 = 6

_CACHE = {}


def _drain_only(self, tick_clock, wait_clock):
    """Tail = DMA/compute drain only; skip the butterfly barrier + sem
    clears (the Bass preamble sem_clears at the start of each execution,
    and NEFF completion already requires every engine queue to finish)."""
    from concourse.tile import ScopedClock

    drain_inst = self.nc.sync.drain()
    wait_clock.add_sem_waits(
        drain_inst.ins, ScopedClock({None: tick_clock.global_clock})
    )
    popped = self.nc._tile_sem_poison_stack.pop()
    assert popped is self._sem_poison


def _build(reps=1):
    import concourse.mybir as mybir
    import concourse.tile as tile
    from concourse import bacc

    nc = bacc.Bacc("TRN2", target_bir_lowering=False, debug=False, num_devices=N_CORES)
    bf16 = mybir.dt.bfloat16
    f32 = mybir.dt.float32

    f8 = mybir.dt.float8e4
    xT_ext = nc.declare_dram_parameter("xT", [128, K_TILES * 1024], bf16, isOutput=False)
    w_ext = nc.declare_dram_parameter("w", [128, K_TILES * 512], bf16, isOutput=False)
    x8_ext = nc.declare_dram_parameter("x8", [128, K_FP8 * 1024], f8, isOutput=False)
    w8_ext = nc.declare_dram_parameter("w8", [128, K_FP8 * 512], f8, isOutput=False)
    b_ext = nc.declare_dram_parameter("bias", [128, N_TILES], f32, isOutput=False)
    out_ext = nc.declare_dram_parameter("out", [128, N_TILES * BATCH], bf16, isOutput=True)

    tc_outer = tile.TileContext(nc)
    try:
        # verify the internals _drain_only touches exist in this concourse
        from concourse.tile import ScopedClock  # noqa: F401

        assert hasattr(tc_outer, "_drain_and_barrier")
        assert hasattr(nc, "_tile_sem_poison_stack")
        tc_outer._drain_and_barrier = types.MethodType(_drain_only, tc_outer)
    except Exception:
        pass  # stock barrier exit: ~4us slower, still correct
    with tc_outer as tc:
        with (
            tc.tile_pool(name="consts", bufs=1) as cpool,
            tc.tile_pool(name="xk", bufs=1) as xpool,
            tc.tile_pool(name="wk", bufs=1) as wpool,
            tc.tile_pool(name="osb", bufs=1) as opool,
            tc.tile_pool(name="psum", bufs=1, space="PSUM") as ppool,
        ):
            psum = [
                ppool.tile([128, 512], f32, tag=f"ps{i}", name=f"ps{i}")
                for i in range(N_TILES * M_HALVES)
            ]

            # PE warm-up against the HAM cold clock: short N=128 matmuls keep
            # the PE-busy window covered until the first data chunk lands.
            # NO memset — the warm-ups read a RAW (untracked, uninitialized)
            # SBUF tensor and start with zero dependencies. Any NaN garbage
            # lands in psum bank 0 whose has_written bits the first real
            # start=True matmul clears (cleared bits force overwrite).
            warm = nc.alloc_sbuf_tensor("warm_raw", [128, 128], bf16)
            for _ in range(5):
                nc.tensor.matmul(
                    psum[0][:, :128], warm[:, 0:128], warm[:, 0:128],
                    start=True, stop=True,
                )

            tbl_warm = cpool.tile([128, 1], f32)
            bias_sb = cpool.tile([128, N_TILES], f32)

            xts = xpool.tile([128, K_TILES * 1024], bf16, name="xts")
            wts = wpool.tile([128, K_TILES * 512], bf16, name="wts")
            x8 = xpool.tile([128, K_FP8, 1024], f8, name="x8")
            w8 = wpool.tile([128, K_FP8, 512], f8, name="w8")
            osb = opool.tile([128, N_TILES * BATCH], bf16, name="osb")

            # interleave x/w chunks in k order, alternating HWDGE rings;
            # fine-grained first chunks (early PE start). The head is split
            # so the FIRST matmul — an N=256 half of bank (0,0) — waits only
            # on a 64 KB x transfer and a 32 KB w transfer on parallel
            # rings (~0.5 us), instead of two 128 KB ones.
            F = K_FP8  # bf16 stream starts at k-tile K_FP8
            chunks = [
                ("x", F * 1024, F * 1024 + 256), ("w", F * 512, F * 512 + 128),
                ("x", F * 1024 + 256, F * 1024 + 512),
                ("w", F * 512 + 128, (F + 1) * 512),
                ("x", F * 1024 + 512, (F + 1) * 1024),
                ("w", (F + 1) * 512, (F + 2) * 512),
            ]
            xbounds = [F + 1, F + 2, F + 3, F + 4, F + 5, F + 6] + list(
                range(F + 8, K_TILES + 1, 2)
            )
            wbounds = [F + 2, F + 3, F + 4, F + 6, F + 8] + list(
                range(F + 12, K_TILES + 1, 4)
            )
            # the strided tails must terminate exactly at K_TILES or the
            # last k-tiles would never be DMA'd
            if xbounds[-1] != K_TILES:
                xbounds.append(K_TILES)
            if wbounds[-1] != K_TILES:
                wbounds.append(K_TILES)
            xi = wi = 0
            while xi < len(xbounds) - 1 or wi < len(wbounds) - 1:
                kx = xbounds[xi] if xi < len(xbounds) - 1 else K_TILES
                kw = wbounds[wi] if wi < len(wbounds) - 1 else K_TILES
                if kw < kx and wi < len(wbounds) - 1:
                    chunks.append(("w", wbounds[wi] * 512, wbounds[wi + 1] * 512))
                    wi += 1
                else:
                    chunks.append(("x", xbounds[xi] * 1024, xbounds[xi + 1] * 1024))
                    xi += 1
            # fp8 operands (k-tiles 0..K_FP8-1, bounds in k-subtile units):
            # consumed by phase 2, streamed behind the bf16 stream
            fp8_chunks = [
                ("x8", 0, 2), ("w8", 0, 3), ("x8", 2, 4),
                ("w8", 3, K_FP8), ("x8", 4, K_FP8),
            ]

            def emit_chunk(i, kind, clo, chi):
                eng = nc.sync if i % 2 == 0 else nc.scalar
                if kind == "x":
                    eng.dma_start(xts[:, clo:chi], xT_ext[:, clo:chi])
                elif kind == "w":
                    eng.dma_start(wts[:, clo:chi], w_ext[:, clo:chi])
                elif kind == "x8":
                    eng.dma_start(x8[:, clo:chi, :], x8_ext[:, clo * 1024 : chi * 1024])
                else:
                    eng.dma_start(w8[:, clo:chi, :], w8_ext[:, clo * 512 : chi * 512])

            # split: "head" chunks cover k < K_STAG (prefetchable during the
            # PREVIOUS body's phase 2); "rest" chunks (bf16 k >= K_STAG and
            # all fp8 — both read by the previous body's phase 2) go after
            # the final outs. fp8 first: their WAR clears ~4 us earlier.
            head_chunks, rest_chunks = [], []
            for i, (kind, clo, chi) in enumerate(chunks):
                lim = K_STAG * (1024 if kind == "x" else 512)
                (head_chunks if chi <= lim else rest_chunks).append((i, kind, clo, chi))
            rest_chunks = [
                (len(chunks) + j, kind, clo, chi)
                for j, (kind, clo, chi) in enumerate(fp8_chunks)
            ] + rest_chunks

            def mm(k, nt, mh, stop=False):
                nc.tensor.matmul(
                    psum[nt * M_HALVES + mh][:, :],
                    wts[:, k * 512 + nt * 128 : k * 512 + (nt + 1) * 128],
                    xts[:, k * 1024 + mh * 512 : k * 1024 + (mh + 1) * 512],
                    start=(k == K_FP8),
                    stop=stop,
                )

            def mm8(kp, nt, mh):
                # fp8e4m3 DoubleRow: one pass covers k-tiles kp, kp+1 (the
                # 3D [128, 2, n] APs pair k-subtiles per PE cell); accumulates
                # into the bank mid-group (start/stop live on the bf16 ends)
                nc.tensor.matmul(
                    psum[nt * M_HALVES + mh][:, :],
                    w8[:, kp : kp + 2, nt * 128 : (nt + 1) * 128],
                    x8[:, kp : kp + 2, mh * 512 : (mh + 1) * 512],
                    start=False,
                    stop=False,
                    perf_mode=mybir.MatmulPerfMode.DoubleRow,
                )

            def epilogue(nt, mh, lo, hi):
                # bias+relu from psum bank (nt,mh) into the packed out tile.
                # ALL epilogues run on DVE: the DVE queue carries nothing
                # else, so its FIFO order always matches readiness order and
                # an epilogue can never sit behind a DMA waiting on a
                # semaphore (ACT/SP queues carry the input streams and would
                # head-of-line-block drains behind next-body chunk WARs).
                dst = osb[:, nt * 1024 + mh * 512 + lo : nt * 1024 + mh * 512 + hi]
                nc.vector.tensor_scalar(
                    dst,
                    psum[nt * M_HALVES + mh][:, lo:hi],
                    bias_sb[:, nt : nt + 1],
                    0.0,
                    mybir.AluOpType.add,
                    mybir.AluOpType.max,
                )

            def emit_out(c0, c1):
                # out-DMAs ride the Pool/GPSIMD SWDGE queue: it is otherwise
                # idle, so drains never contend with (or block) the input
                # rings; the ~1us SWDGE setup hides under the matmul stream.
                nc.gpsimd.dma_start(out_ext[:, c0:c1], osb[:, c0:c1])

            for r in range(reps):
                if r == 0:
                    for i, kind, clo, chi in head_chunks:
                        emit_chunk(i, kind, clo, chi)
                    for i, kind, clo, chi in rest_chunks:
                        emit_chunk(i, kind, clo, chi)
                    # bias (2 KB) rides behind the input stream — needed
                    # only by the first epilogue ~14 us before stream end
                    nc.sync.dma_start(bias_sb[:, :], b_ext[:, :])
                    # trigger the Relu act-table load now (ACT is idle during
                    # the stream); bacc hoists LoadActFuncSet before this
                    # instruction, keeping the ~1.3us load off the final
                    # epilogue-half critical path
                    nc.scalar.activation(
                        tbl_warm[:, :], warm[:, 0:1],
                        mybir.ActivationFunctionType.Relu,
                    )

                # k=K_FP8 (first bf16 k-tile), mh=0 first across all nt;
                # bank (0,0) starts as two N=256 halves gated on just the
                # first 64 KB x chunk. start=True clears has_written for the
                # WHOLE bank, so only the first half sets it; the second
                # half runs start=False and overwrites its cleared region.
                kf = K_FP8
                nc.tensor.matmul(
                    psum[0][:, 0:256],
                    wts[:, kf * 512 : kf * 512 + 128],
                    xts[:, kf * 1024 : kf * 1024 + 256],
                    start=True, stop=False,
                )
                nc.tensor.matmul(
                    psum[0][:, 256:512],
                    wts[:, kf * 512 : kf * 512 + 128],
                    xts[:, kf * 1024 + 256 : kf * 1024 + 512],
                    start=False, stop=False,
                )
                for nt in range(1, N_TILES):
                    mm(kf, nt, 0)
                for nt in range(N_TILES):
                    mm(kf, nt, 1)

                # phase 1: k-major over all banks — keeps the PE stream dense
                # while DMA feeds k-tiles. mh-inner so each LDWEIGHTS serves
                # two matmuls.
                for k in range(K_FP8 + 1, K_STAG):
                    for nt in range(N_TILES):
                        for mh in range(M_HALVES):
                            mm(k, nt, mh)

                if r < reps - 1:
                    # rep-boundary prefetch BEFORE this body's outs hit the
                    # rings: the sequencers process DMAs in FIFO order and an
                    # out-DMA waiting on a late epilogue would block the next
                    # body's head behind it. These chunks' WAR deps (this
                    # body's phase-1 reads of k<K_STAG) clear midstream, so
                    # they land long before the boundary.
                    for i, kind, clo, chi in head_chunks:
                        emit_chunk(i, kind, clo, chi)

                # phase 2: bank-pair-major — pair nt runs its last 8 k-tiles
                # (mh pairs share LDWEIGHTS), drains both banks (fused
                # bias+relu, ACT for mh=0 / DVE for mh=1) into the packed
                # tile, and fires ONE 256KB out-DMA. Pair completions are
                # ~3.4 us apart, so every drain pipelines under the
                # remaining matmul stream with the input rings idle.
                for nt in range(N_TILES - 1):
                    for kp in range(0, K_FP8, 2):
                        mm8(kp, nt, 0)
                        mm8(kp, nt, 1)
                    for k in range(K_STAG, K_TILES):
                        mm(k, nt, 0, stop=(k == K_TILES - 1))
                        mm(k, nt, 1, stop=(k == K_TILES - 1))
                    epilogue(nt, 0, 0, 512)
                    epilogue(nt, 1, 0, 512)
                    emit_out(nt * 1024, (nt + 1) * 1024)

                # final pair (nt=3): bank6 drains 8 matmuls (~1.7us) early;
                # bank7's epilogue + 128KB DMA are the only exposed tail.
                nt = N_TILES - 1
                for kp in range(0, K_FP8, 2):
                    mm8(kp, nt, 0)
                for k in range(K_STAG, K_TILES):
                    mm(k, nt, 0, stop=(k == K_TILES - 1))
                epilogue(nt, 0, 0, 512)
                emit_out(nt * 1024, nt * 1024 + 512)  # D: bank6
                for kp in range(0, K_FP8, 2):
                    mm8(kp, nt, 1)
                if r < reps - 1:
                    for k in range(K_STAG, K_TILES):
                        mm(k, nt, 1, stop=(k == K_TILES - 1))
                    epilogue(nt, 1, 0, 512)
                    emit_out(nt * 1024 + 512, (nt + 1) * 1024)  # E: bank7
                else:
                    # last body: minimal exposed tail. k=31 is split into two
                    # N=256 halves so the lo half of the bank retires ~107ns
                    # early; the epilogue halves run in PARALLEL on DVE (lo)
                    # and ACT (hi, fused bias+relu via the preloaded act
                    # table); the two 64KB out-DMAs drain concurrently on the
                    # Pool(SWDGE) and SP(HWDGE) queues. Both queues are
                    # empty at this point in the final body, so nothing can
                    # head-of-line block them.
                    for k in range(K_STAG, K_TILES - 1):
                        mm(k, nt, 1, stop=False)
                    c7 = nt * 1024 + 512
                    nc.tensor.matmul(
                        psum[nt * M_HALVES + 1][:, 0:256],
                        wts[:, 31 * 512 + nt * 128 : 31 * 512 + (nt + 1) * 128],
                        xts[:, 31 * 1024 + 512 : 31 * 1024 + 768],
                        start=False, stop=False,
                    )
                    nc.tensor.matmul(
                        psum[nt * M_HALVES + 1][:, 256:512],
                        wts[:, 31 * 512 + nt * 128 : 31 * 512 + (nt + 1) * 128],
                        xts[:, 31 * 1024 + 768 : 31 * 1024 + 1024],
                        start=False, stop=True,
                    )
                    epilogue(nt, 1, 0, 256)  # lo on DVE
                    nc.scalar.activation(
                        osb[:, c7 + 256 : c7 + 512],
                        psum[nt * M_HALVES + 1][:, 256:512],
                        mybir.ActivationFunctionType.Relu,
                        bias=bias_sb[:, nt : nt + 1],
                    )
                    nc.gpsimd.dma_start(
                        out_ext[:, c7 : c7 + 256], osb[:, c7 : c7 + 256]
                    )
                    nc.sync.dma_start(
                        out_ext[:, c7 + 256 : c7 + 512], osb[:, c7 + 256 : c7 + 512]
                    )
                if r < reps - 1:
                    # k>=K_STAG chunks are blocked by this body's phase-2
                    # readers anyway; they go behind the final outs.
                    for i, kind, clo, chi in rest_chunks:
                        emit_chunk(i, kind, clo, chi)

    nc.compile()
    return nc


def _get_nc():
    if "nc" not in _CACHE:
        _CACHE["nc"] = _build()
    return _CACHE["nc"]


def prep_in_maps(x, values, bias, rows, cols):
    x = np.asarray(x, np.float32)
    values = np.asarray(values, np.float32)
    bias = np.asarray(bias, np.float32)
    rows = np.asarray(rows)
    cols = np.asarray(cols)

    # densify via bincount (vectorized scatter-add; duplicates accumulate)
    flat = rows.astype(np.int64) * UNITS + cols.astype(np.int64)
    W = np.bincount(flat, weights=values.astype(np.float64), minlength=D_IN * UNITS)
    W = W.reshape(D_IN, UNITS).astype(np.float32)

    # partition-major xT: xT_pm[p, k*1024 + m] = x[m, k*128 + p]
    xT32 = np.ascontiguousarray(x.T)  # [D_IN, BATCH]
    xT16 = xT32.astype(ml_dtypes.bfloat16)
    xT_pm = np.ascontiguousarray(
        xT16.reshape(K_TILES, 128, BATCH).transpose(1, 0, 2).reshape(128, K_TILES * BATCH)
    )
    # fp8 part: k-tiles 0..K_FP8-1 (values well inside e4m3 range, no clip)
    x8 = xT32[: K_FP8 * 128].astype(ml_dtypes.float8_e4m3)
    x8_pm = np.ascontiguousarray(
        x8.reshape(K_FP8, 128, BATCH).transpose(1, 0, 2).reshape(128, K_FP8 * BATCH)
    )
    W16 = W.astype(ml_dtypes.bfloat16)
    W8 = W[: K_FP8 * 128].astype(ml_dtypes.float8_e4m3)

    in_maps = []
    for i in range(N_CORES):
        w_shard = W16[:, i * N_SHARD : (i + 1) * N_SHARD]  # [D_IN, 512]
        # partition-major W: w_pm[p, k*512 + n] = W[k*128 + p, n0 + n]
        w_pm = np.ascontiguousarray(
            w_shard.reshape(K_TILES, 128, N_SHARD)
            .transpose(1, 0, 2)
            .reshape(128, K_TILES * N_SHARD)
        )
        w8_shard = W8[:, i * N_SHARD : (i + 1) * N_SHARD]  # [K_FP8*128, 512]
        w8_pm = np.ascontiguousarray(
            w8_shard.reshape(K_FP8, 128, N_SHARD)
            .transpose(1, 0, 2)
            .reshape(128, K_FP8 * N_SHARD)
        )
        b_shard = np.ascontiguousarray(
            bias[i * N_SHARD : (i + 1) * N_SHARD].reshape(N_TILES, 128).T
        )
        in_maps.append(
            {"xT": xT_pm, "w": w_pm, "x8": x8_pm, "w8": w8_pm, "bias": b_shard}
        )
    return in_maps


def kernel(x, values, bias, rows, cols):
    from concourse.bass_utils import run_bass_kernel_spmd

    in_maps = prep_in_maps(x, values, bias, rows, cols)
    nc = _get_nc()
    res = run_bass_kernel_spmd(nc, in_maps, list(range(N_CORES)))
    out = np.empty((BATCH, UNITS), np.float32)
    for i in range(N_CORES):
        # packed [128, nt*1024 + b] -> out[b, 512*i + nt*128 + p]
        arr = res.results[i]["out"].astype(np.float32).reshape(128, N_TILES, BATCH)
        out[:, i * N_SHARD : (i + 1) * N_SHARD] = (
            arr.transpose(1, 0, 2).reshape(N_SHARD, BATCH).T
        )
    return out
